# revision 1
# baseline (speedup 1.0000x reference)
"""Multi-head attention (B=8, N=1024, DIM=768, H=12) on 8 Trainium2 cores.

Sharding: data-parallel over batch — core b computes batch element b.
Per-core kernel: qkv = x @ w_qkv^T; per-head softmax(q k^T / sqrt(dh)) @ v;
out proj + bias. All matmuls in float32r (TF32-like) mode.

Layout strategy (per core, x_b is [N, D]):
  - host supplies x^T [D, N], w_qkv^T (split qk / v parts), w_proj^T
  - QKV phase: qkT [e, n] feature-major for q,k;  v token-major [n, dh]
    packed per head as lhsT tiles [128, 128] = [v_h | ones] (ones block
    yields softmax denominators for free during attn@V)
  - scoresT[j, i] = k^T(lhsT) x q^T(rhs) -> PSUM; ACT exp (scale folded);
    no max-subtraction (scores ~ N(0,1), max << 80)
  - attn@V: out'[0:64] = unnormalized out^T, out'[64:128] = denom bcast
  - normalize: reciprocal_approx_fast + tensor_tensor mult -> outT [d, n]
  - proj: y[n, f] = outT(lhsT) x wprojT(rhs) + bias (bias tile broadcast
    across partitions via DMA, fused into PSUM evacuation add)
"""

import numpy as np

import concourse.bass as bass
import concourse.mybir as mybir
import concourse.tile as tile
from concourse import bacc
from concourse.alu_op_type import AluOpType
from concourse.bass_utils import run_bass_kernel_spmd

B, N, DIM, H = 8, 1024, 768, 12
DH = DIM // H          # 64
E_QK = 2 * DIM         # 1536
E_V = DIM              # 768
SCALE = DH ** -0.5
NCORES = 8

F32 = mybir.dt.float32
F32R = mybir.dt.float32r

N_TILES = N // 128     # 8
D_CHUNKS = DIM // 128  # 6
QK_TILES = E_QK // 128  # 12
EXP = mybir.ActivationFunctionType.Exp


def build_nc(reps=1, phases="abc", qkv_dt=None, split_dma=False,
             fast_a=False):
    qkv_dt = qkv_dt or F32R
    nc = bacc.Bacc("TRN2", target_bir_lowering=False, debug=False,
                   num_devices=NCORES)

    xT_d = nc.dram_tensor("xT", [DIM, N], qkv_dt, kind="ExternalInput")
    wqk_d = nc.dram_tensor("wqkT", [DIM, E_QK], qkv_dt, kind="ExternalInput")
    wv_d = nc.dram_tensor("wvT", [DIM, E_V], qkv_dt, kind="ExternalInput")
    wp_d = nc.dram_tensor("wpT", [DIM, DIM], F32R, kind="ExternalInput")
    bias_d = nc.dram_tensor("b_proj", [DIM], F32, kind="ExternalInput")
    y_d = nc.dram_tensor("y", [N, DIM], F32, kind="ExternalOutput")

    with tile.TileContext(nc) as tc:
      hoisted = None
      if phases == "amm":
          hpool = tc.alloc_tile_pool(name="hoist", bufs=1)
          xT_h = hpool.tile([128, D_CHUNKS, N], qkv_dt, name="xT_h")
          nc.sync.dma_start(
              xT_h[:], xT_d.ap().rearrange("(p dc) n -> p dc n", dc=D_CHUNKS))
          wv_h = hpool.tile([128, D_CHUNKS, E_V], qkv_dt, name="wv_h")
          nc.sync.dma_start(
              wv_h[:], wv_d.ap().rearrange("(p dc) f -> p dc f", dc=D_CHUNKS))
          wqk_h = hpool.tile([128, D_CHUNKS, E_QK], qkv_dt, name="wqk_h")
          nc.sync.dma_start(
              wqk_h[:], wqk_d.ap().rearrange("(p dc) e -> p dc e", dc=D_CHUNKS))
          hoisted = (xT_h, wv_h, wqk_h)
      _hpool = hpool if hoisted else None
      for _rep in range(reps):
        with tc.tile_pool(name="persist", bufs=1) as persist:
            # ---- persistent tiles (live through proj) ----
            outT = persist.tile([128, D_CHUNKS, N], F32R)      # 24K/part
            bias_bc = persist.tile([128, DIM], F32)            # 3K/part

            nc.gpsimd.dma_start(
                out=bias_bc[:],
                in_=bias_d.ap()[None, :].broadcast_to([128, DIM]),
            )

            with tc.tile_pool(name="qkv_sb", bufs=1) as qkv_sb:
                # ---- tiles live until end of attention ----
                qkT = qkv_sb.tile([128, QK_TILES, N], F32R)        # 48K/part
                vp = qkv_sb.tile([128, N_TILES, H, 128], F32R)     # 48K/part

                # ================= Phase A: QKV projections =================
                skip_mm = (phases == "adma")
                with (
                    tc.tile_pool(name="xpool", bufs=1) as xpool,
                    tc.tile_pool(name="psA", bufs=8 if fast_a else 4,
                                 space="PSUM") as psA,
                ):
                    if hoisted is None:
                        xT = xpool.tile([128, D_CHUNKS, N], qkv_dt)  # 24K/part
                        if split_dma:
                            xr = xT_d.ap().rearrange(
                                "(dc p) n -> p dc n", p=128)
                            for dc in range(D_CHUNKS):
                                nc.sync.dma_start(
                                    xT[:, dc, :], xr[:, dc, :])
                        else:
                            nc.sync.dma_start(
                                xT[:],
                                xT_d.ap().rearrange(
                                    "(p dc) n -> p dc n", dc=D_CHUNKS))
                    else:
                        xT = hoisted[0]

                    # ---- V part: v[n, dh] per head + ones block ----
                    with tc.tile_pool(name="wvpool", bufs=1) as wvpool:
                        if hoisted is None:
                            wv = wvpool.tile([128, D_CHUNKS, E_V], qkv_dt)
                            if split_dma:
                                wvr = wv_d.ap().rearrange(
                                    "(dc p) f -> p dc f", p=128)
                                for dc in range(D_CHUNKS):
                                    nc.sync.dma_start(
                                        wv[:, dc, :], wvr[:, dc, :])
                            else:
                                nc.sync.dma_start(
                                    wv[:],
                                    wv_d.ap().rearrange(
                                        "(p dc) f -> p dc f", dc=D_CHUNKS))
                        else:
                            wv = hoisted[1]
                        if skip_mm:
                            cpool3 = tc.alloc_tile_pool(name="consume3", bufs=1)
                            ct3 = cpool3.tile([128, 128], F32, name="ct3")
                            nc.vector.tensor_copy(
                                ct3[:], wv[:, 0, 0:128].bitcast(F32))
                            nc.sync.dma_start(y_d.ap()[128:256, 0:128], ct3[:])
                            cpool3.release()
                        for jt in (range(N_TILES) if not skip_mm else []):
                            for fc, fw in ((0, 512), (512, 256)):
                                ps = psA.tile([128, fw], F32, tag="psA")
                                for dc in range(D_CHUNKS):
                                    nc.tensor.matmul(
                                        ps[:],
                                        xT[:, dc, jt * 128:(jt + 1) * 128],
                                        wv[:, dc, fc:fc + fw],
                                        start=(dc == 0),
                                        stop=(dc == D_CHUNKS - 1),
                                    )
                                h0, nh = fc // DH, fw // DH
                                nc.vector.tensor_copy(
                                    vp[:, jt, h0:h0 + nh, 0:DH],
                                    ps[:].rearrange("p (h c) -> p h c", c=DH),
                                )
                            # memset on f32r fails walrus ISA check; write
                            # the 1.0f bit pattern through a uint32 view
                            nc.vector.memset(
                                vp[:, jt, :, DH:128].bitcast(mybir.dt.uint32),
                                0x3F800000)

                    # ---- QK part: qkT [e, n] feature-major; head-pair
                    # order (q-tile, k-tile alternating) so attention can
                    # start as soon as the first pair lands ----
                    with tc.tile_pool(name="wqkpool", bufs=1) as wqkpool:
                        if hoisted is None:
                            wqk = wqkpool.tile([128, D_CHUNKS, E_QK], qkv_dt)
                            if split_dma:
                                wqkr = wqk_d.ap().rearrange(
                                    "(dc p) e -> p dc e", p=128)
                                for dc in range(D_CHUNKS):
                                    nc.sync.dma_start(
                                        wqk[:, dc, :], wqkr[:, dc, :])
                            else:
                                nc.sync.dma_start(
                                    wqk[:],
                                    wqk_d.ap().rearrange(
                                        "(p dc) e -> p dc e", dc=D_CHUNKS))
                        else:
                            wqk = hoisted[2]
                        et_order = []
                        for i in range(H // 2):
                            et_order += [i, H // 2 + i]
                        if skip_mm:
                            cpool2 = tc.alloc_tile_pool(name="consume2", bufs=1)
                            ct2 = cpool2.tile([128, 640], F32, name="ct2")
                            nc.vector.tensor_copy(
                                ct2[:, 0:512], xT[:, 0, 0:512].bitcast(F32))
                            nc.vector.tensor_copy(
                                ct2[:, 512:640], wqk[:, 0, 0:128].bitcast(F32))
                            nc.sync.dma_start(y_d.ap()[0:128, 0:640], ct2[:])
                            cpool2.release()
                        for et in (et_order if not skip_mm else []):
                            for ncn in range(2):
                                ps = psA.tile([128, 512], F32, tag="psA")
                                for dc in range(D_CHUNKS):
                                    nc.tensor.matmul(
                                        ps[:],
                                        wqk[:, dc, et * 128:(et + 1) * 128],
                                        xT[:, dc, ncn * 512:(ncn + 1) * 512],
                                        start=(dc == 0),
                                        stop=(dc == D_CHUNKS - 1),
                                    )
                                if fast_a and ncn == 1:
                                    nc.scalar.copy(
                                        qkT[:, et, ncn * 512:(ncn + 1) * 512],
                                        ps[:])
                                else:
                                    nc.vector.tensor_copy(
                                        qkT[:, et, ncn * 512:(ncn + 1) * 512],
                                        ps[:])

                if "b" not in phases:
                    # timing variant: consume tiles so DCE keeps the work
                    with tc.tile_pool(name="consume", bufs=1) as consume:
                        ct1 = consume.tile([128, 640], F32)
                        if phases == "adma":
                            nc.vector.memset(ct1[:].bitcast(mybir.dt.uint32), 0)
                        else:
                            nc.vector.tensor_copy(
                                ct1[:, 0:512], qkT[:, 0, 0:512].bitcast(F32))
                            nc.vector.tensor_copy(
                                ct1[:, 512:640], vp[:, 0, 0, :].bitcast(F32))
                        nc.sync.dma_start(y_d.ap()[0:128, 0:640], ct1[:])
                    continue

                # ========== Phases B+C: attention + projection ==========
                # psC allocated alongside B pools (2+4+2 = 8 PSUM banks) so
                # projection matmuls fill PE gaps while ACT paces softmax.
                with (
                    tc.tile_pool(name="cpool", bufs=1) as cpool,
                    tc.tile_pool(name="ypool", bufs=2) as ypool,
                    tc.tile_pool(name="psC", bufs=2, space="PSUM") as psC,
                ):
                    wp = cpool.tile([128, D_CHUNKS, DIM], F32R)    # 18K/part
                    nc.sync.dma_start(
                        wp[:], wp_d.ap().rearrange("(dc p) f -> p dc f", p=128))

                    with (
                        tc.tile_pool(name="ptpool", bufs=3) as ptpool,
                        tc.tile_pool(name="recpool", bufs=2) as recpool,
                        tc.tile_pool(name="psS", bufs=2, space="PSUM") as psS,
                        tc.tile_pool(name="psO", bufs=1, space="PSUM") as psO,
                    ):
                        for h in range(H):
                            base = 64 * (h % 2)
                            q_et = h // 2
                            k_et = H // 2 + h // 2
                            ps_o = psO.tile([128, N], F32, tag="ps_o")
                            for jt in range(N_TILES):
                                ps_s = psS.tile([128, N], F32, tag="ps_s")
                                for ic in range(2):
                                    nc.tensor.matmul(
                                        ps_s[:, ic * 512:(ic + 1) * 512],
                                        qkT[base:base + DH, k_et,
                                            jt * 128:(jt + 1) * 128],
                                        qkT[base:base + DH, q_et,
                                            ic * 512:(ic + 1) * 512],
                                        start=True, stop=True,
                                    )
                                pt = ptpool.tile([128, N], F32R, tag="pt")
                                nc.scalar.activation(
                                    pt[:], ps_s[:], EXP, scale=SCALE)
                                for ic in range(2):
                                    nc.tensor.matmul(
                                        ps_o[:, ic * 512:(ic + 1) * 512],
                                        vp[:, jt, h, :],
                                        pt[:, ic * 512:(ic + 1) * 512],
                                        start=(jt == 0),
                                        stop=(jt == N_TILES - 1),
                                    )
                            # reciprocal_approx_fast needs SBUF input at
                            # partition base 0 — normalize at base 0, DVE
                            # handles the out-base shift on the final mult
                            den = recpool.tile([64, N], F32, tag="den")
                            nc.vector.tensor_copy(den[0:64, :],
                                                  ps_o[64:128, :])
                            rec = recpool.tile([64, N], F32, tag="rec")
                            nc.vector.reciprocal_approx_fast(
                                rec[0:64, :], den[0:64, :])
                            nc.vector.tensor_tensor(
                                outT[base:base + 64, h // 2, :],
                                ps_o[0:64, :], rec[0:64, :],
                                op=AluOpType.mult)

                    # ---- projection; emitted last, scheduled into gaps ----
                    if "c" not in phases:
                        yt0 = ypool.tile([128, DIM], F32, tag="yt")
                        nc.vector.tensor_copy(
                            yt0[:, 0:DIM], outT[:, 0, 0:DIM].bitcast(F32))
                        nc.sync.dma_start(y_d.ap()[0:128, :], yt0[:])
                    for nt in (range(N_TILES) if "c" in phases else []):
                        yt = ypool.tile([128, DIM], F32, tag="yt")
                        for fc, fw in ((0, 512), (512, 256)):
                            ps = psC.tile([128, fw], F32, tag="psC")
                            for dc in range(D_CHUNKS):
                                nc.tensor.matmul(
                                    ps[:],
                                    outT[:, dc, nt * 128:(nt + 1) * 128],
                                    wp[:, dc, fc:fc + fw],
                                    start=(dc == 0), stop=(dc == D_CHUNKS - 1),
                                )
                            nc.vector.tensor_tensor(
                                yt[:, fc:fc + fw], ps[:],
                                bias_bc[:, fc:fc + fw], op=AluOpType.add)
                        nc.sync.dma_start(
                            y_d.ap().rearrange("(nt p) f -> p nt f",
                                               p=128)[:, nt, :],
                            yt[:])

      if phases == "amm":
          _hpool.release()
    nc.compile()
    return nc


_NC_CACHE = None


def _get_nc():
    global _NC_CACHE
    if _NC_CACHE is None:
        _NC_CACHE = build_nc()
    return _NC_CACHE


_RUNNER_CACHE = None


def _get_runner():
    """Cached jitted shard_map runner over 8 cores (mirrors
    bass2jax.run_bass_via_pjrt, but reusable across calls for timing)."""
    global _RUNNER_CACHE
    if _RUNNER_CACHE is not None:
        return _RUNNER_CACHE
    import jax
    from jax.experimental.shard_map import shard_map
    from jax.sharding import Mesh, PartitionSpec
    from concourse import bass2jax, mybir as _mb

    nc = _get_nc()
    bass2jax.install_neuronx_cc_hook()

    partition_name = (nc.partition_id_tensor.name
                      if nc.partition_id_tensor else None)
    in_names, out_names, out_avals, zero_outs = [], [], [], []
    for alloc in nc.m.functions[0].allocations:
        if not isinstance(alloc, _mb.MemoryLocationSet):
            continue
        name = alloc.memorylocations[0].name
        if alloc.kind == "ExternalInput":
            if name != partition_name:
                in_names.append(name)
        elif alloc.kind == "ExternalOutput":
            out_names.append(name)
            out_avals.append(jax.core.ShapedArray(
                tuple(alloc.tensor_shape), _mb.dt.np(alloc.dtype)))
            zero_outs.append(np.zeros(
                tuple(alloc.tensor_shape), _mb.dt.np(alloc.dtype)))

    n_params = len(in_names)
    all_in_names = in_names + out_names
    if partition_name is not None:
        all_in_names = all_in_names + [partition_name]

    def _body(*args):
        operands = list(args)
        if partition_name is not None:
            operands.append(bass2jax.partition_id_tensor())
        outs = bass2jax._bass_exec_p.bind(
            *operands,
            out_avals=tuple(out_avals),
            in_names=tuple(all_in_names),
            out_names=tuple(out_names),
            lowering_input_output_aliases=(),
            sim_require_finite=True,
            sim_require_nnan=True,
            nc=nc,
        )
        return tuple(outs)

    devices = jax.devices()[:NCORES]
    mesh = Mesh(np.asarray(devices), ("core",))
    n_outs = len(out_names)
    sharded = jax.jit(
        shard_map(
            _body, mesh=mesh,
            in_specs=(PartitionSpec("core"),) * (n_params + n_outs),
            out_specs=(PartitionSpec("core"),) * n_outs,
            check_rep=False,
        ),
        donate_argnums=tuple(range(n_params, n_params + n_outs)),
        keep_unused=True,
    )
    _RUNNER_CACHE = (sharded, in_names, out_names, out_avals, zero_outs)
    return _RUNNER_CACHE


def _prep_inputs(x, w_qkv, w_proj, b_proj, qkv_bf16=False):
    x = np.ascontiguousarray(np.asarray(x, dtype=np.float32))
    w_qkv = np.asarray(w_qkv, dtype=np.float32)
    w_proj = np.asarray(w_proj, dtype=np.float32)
    b_proj = np.ascontiguousarray(np.asarray(b_proj, dtype=np.float32))

    xT = np.ascontiguousarray(x.transpose(0, 2, 1))              # [B, D, N]
    wqkT = np.ascontiguousarray(w_qkv[:E_QK].T)                  # [D, 2D]
    wvT = np.ascontiguousarray(w_qkv[E_QK:].T)                   # [D, D]
    wpT = np.ascontiguousarray(w_proj.T)                         # [D, D]
    if qkv_bf16:
        import ml_dtypes
        xT = xT.astype(ml_dtypes.bfloat16)
        wqkT = wqkT.astype(ml_dtypes.bfloat16)
        wvT = wvT.astype(ml_dtypes.bfloat16)
    per_core = {"xT": None, "wqkT": wqkT, "wvT": wvT, "wpT": wpT,
                "b_proj": b_proj}

    def core_map(b):
        m = dict(per_core)
        m["xT"] = xT[b]
        return m

    return [core_map(b) for b in range(NCORES)]


def _run(in_maps):
    sharded, in_names, out_names, out_avals, zero_outs = _get_runner()
    concat_in = [
        np.concatenate([np.asarray(in_maps[c][n]) for c in range(NCORES)],
                       axis=0)
        for n in in_names
    ]
    concat_zeros = [
        np.zeros((NCORES * z.shape[0], *z.shape[1:]), z.dtype)
        for z in zero_outs
    ]
    out_arrs = sharded(*concat_in, *concat_zeros)
    yi = out_names.index("y")
    return np.asarray(out_arrs[yi]).reshape(NCORES, N, DIM)


def kernel(x, w_qkv, w_proj, b_proj):
    in_maps = _prep_inputs(x, w_qkv, w_proj, b_proj)
    res = run_bass_kernel_spmd(_get_nc(), in_maps,
                               core_ids=list(range(NCORES)))
    return np.stack([res.results[b]["y"] for b in range(NCORES)], axis=0)



# revision 2
# speedup vs baseline: 26.6237x; 26.6237x over previous
"""Multi-head attention (B=8, N=1024, DIM=768, H=12) on 8 Trainium2 cores.

Sharding: data-parallel over batch — core b computes batch element b.
Per-core kernel: qkv = x @ w_qkv^T; per-head softmax(q k^T / sqrt(dh)) @ v;
out proj + bias.  All matmuls in float32r (TF32-like) mode.

Layout (per core, x_b is [N, D]):
  - host supplies x^T [D, N], w_qkv^T (split qk / v parts), w_proj^T
  - QKV phase: qkT [e, n] feature-major for q,k; v token-major [n, dh]
    packed per head as lhsT tiles [128, 128] = [v_h | ones] (ones block
    yields softmax denominators for free during attn@V)
  - scoresT[j, i] = k^T(lhsT) x q^T(rhs) -> PSUM; ACT exp (scale folded);
    no max-subtraction (scores ~ N(0,1), max << 80)
  - attn@V: out'[0:64] = unnormalized out^T, out'[64:128] = denom
  - normalize: reciprocal_approx_fast + tensor_tensor mult -> outT [d, n]
  - proj: y[n, f] = outT(lhsT) x wprojT(rhs) + bias

HW-tuned schedule (defaults = fastest measured on-device config):
  - vp ones memset on gpsimd (Pool engine idle otherwise)
  - per-head normalize deferred one head (software pipelining)
  - all input DMAs on the SP HWDGE queue; PSUM: psA 4 banks /
    psS 2x2 + psO 1x2 + psC 2x1 banks.  The cost-model-preferred
    variants (ACT-queue weight DMAs, split xT DMA, psO double-
    buffering) all measured SLOWER on real hardware; `revert` flags
    keep both schedules buildable for future bisection.
"""

import numpy as np

import concourse.bass as bass
import concourse.mybir as mybir
import concourse.tile as tile
from concourse import bacc
from concourse.alu_op_type import AluOpType
from concourse.bass_utils import run_bass_kernel_spmd

B, N, DIM, H = 8, 1024, 768, 12
DH = DIM // H          # 64
E_QK = 2 * DIM         # 1536
E_V = DIM              # 768
SCALE = DH ** -0.5
NCORES = 8

F32 = mybir.dt.float32
F32R = mybir.dt.float32r

N_TILES = N // 128     # 8
D_CHUNKS = DIM // 128  # 6
QK_TILES = E_QK // 128  # 12
EXP = mybir.ActivationFunctionType.Exp

# Schraudolph fast-exp constants: exp(s*SCALE) ~= bitcast_f32(round(
# s * (2^23/ln2 * SCALE) + (127 - 0.0437)*2^23)).  ~1.8% mean rel err;
# applied to a subset of softmax tiles to offload the ACT engine.
FEXP_A = float(2 ** 23 / np.log(2) * SCALE)
FEXP_C = float((127.0 - 0.043677) * 2 ** 23)
I32 = mybir.dt.int32
FEXP_JT = (3, 4, 5)


TIME_REPS = 16         # in-NEFF repetitions used by test.py's timing


def build_nc(reps=1, phases="abc", qkv_dt=None, fexp_mod=0, hw_loop=0,
             revert=("v1psum", "spdma")):
    """hw_loop>0: wrap ONE rep body in a tc.For_i hardware loop executing
    `hw_loop` iterations (compile cost of 1 rep, runtime of hw_loop reps);
    `reps` is the python-unrolled count (ignored when hw_loop is set).
    revert: any of {'spdma','dvememset','nodefer','v1psum'} to undo one
    v2 schedule change (HW bisection)."""
    qkv_dt = qkv_dt or F32R
    spdma = "spdma" in revert          # all input DMAs on SP, xT unsplit
    dvememset = "dvememset" in revert  # vp ones memset on DVE per jt
    nodefer = "nodefer" in revert      # normalize at head end
    v1psum = "v1psum" in revert        # psO bufs=1, psC pool alongside
    nc = bacc.Bacc("TRN2", target_bir_lowering=False, debug=False,
                   num_devices=NCORES)

    xT_d = nc.dram_tensor("xT", [DIM, N], qkv_dt, kind="ExternalInput")
    wqk_d = nc.dram_tensor("wqkT", [DIM, E_QK], qkv_dt, kind="ExternalInput")
    wv_d = nc.dram_tensor("wvT", [DIM, E_V], qkv_dt, kind="ExternalInput")
    wp_d = nc.dram_tensor("wpT", [DIM, DIM], F32R, kind="ExternalInput")
    bias_d = nc.dram_tensor("b_proj", [DIM], F32, kind="ExternalInput")
    y_d = nc.dram_tensor("y", [N, DIM], F32, kind="ExternalOutput")

    from contextlib import nullcontext
    with tile.TileContext(nc) as tc:
      loop_ctx = tc.For_i(0, hw_loop) if hw_loop else nullcontext()
      with loop_ctx:
       for _rep in range(1 if hw_loop else reps):
        with tc.tile_pool(name="persist", bufs=1) as persist:
            # ---- persistent tiles (live through proj) ----
            outT = persist.tile([128, D_CHUNKS, N], F32R)      # 24K/part
            bias_bc = persist.tile([128, DIM], F32)            # 3K/part

            nc.gpsimd.dma_start(
                out=bias_bc[:],
                in_=bias_d.ap()[None, :].broadcast_to([128, DIM]),
            )

            with tc.tile_pool(name="qkv_sb", bufs=1) as qkv_sb:
                # ---- tiles live until end of attention ----
                qkT = qkv_sb.tile([128, QK_TILES, N], F32R)        # 48K/part
                vp = qkv_sb.tile([128, N_TILES, H, 128], F32R)     # 48K/part

                # ================= Phase A: QKV projections =================
                with (
                    tc.tile_pool(name="xpool", bufs=1) as xpool,
                    tc.tile_pool(name="wvpool", bufs=1) as wvpool,
                    tc.tile_pool(name="wqkpool", bufs=1) as wqkpool,
                    tc.tile_pool(name="psA", bufs=4, space="PSUM") as psA,
                ):
                    # SP queue: xT in 2 n-chunks (V/QK start on chunk 0)
                    xT = xpool.tile([128, D_CHUNKS, N], qkv_dt)  # 24K/part
                    xr = xT_d.ap().rearrange("(p dc) n -> p dc n",
                                             dc=D_CHUNKS)
                    if spdma:
                        nc.sync.dma_start(xT[:], xr)
                    else:
                        nc.sync.dma_start(xT[:, :, 0:512], xr[:, :, 0:512])
                        nc.sync.dma_start(xT[:, :, 512:N], xr[:, :, 512:N])
                    # ACT queue: wv then wqk then wp (all during phase A)
                    weng = nc.sync if spdma else nc.scalar
                    wv = wvpool.tile([128, D_CHUNKS, E_V], qkv_dt)
                    weng.dma_start(
                        wv[:],
                        wv_d.ap().rearrange("(p dc) f -> p dc f",
                                            dc=D_CHUNKS))
                    wqk = wqkpool.tile([128, D_CHUNKS, E_QK], qkv_dt)
                    weng.dma_start(
                        wqk[:],
                        wqk_d.ap().rearrange("(p dc) e -> p dc e",
                                             dc=D_CHUNKS))

                    # ---- V part: v[n, dh] per head + ones block ----
                    # ones block via gpsimd (idle Pool engine); memset on
                    # f32r fails walrus ISA check; write the 1.0f bit
                    # pattern through a uint32 view
                    if not dvememset:
                        nc.gpsimd.memset(
                            vp[:, :, :, DH:128].bitcast(mybir.dt.uint32),
                            0x3F800000)
                    for jt in range(N_TILES):
                        for fc, fw in ((0, 512), (512, 256)):
                            ps = psA.tile([128, fw], F32, tag="psA")
                            for dc in range(D_CHUNKS):
                                nc.tensor.matmul(
                                    ps[:],
                                    xT[:, dc, jt * 128:(jt + 1) * 128],
                                    wv[:, dc, fc:fc + fw],
                                    start=(dc == 0),
                                    stop=(dc == D_CHUNKS - 1),
                                )
                            h0, nh = fc // DH, fw // DH
                            nc.vector.tensor_copy(
                                vp[:, jt, h0:h0 + nh, 0:DH],
                                ps[:].rearrange("p (h c) -> p h c", c=DH),
                            )
                        if dvememset:
                            nc.vector.memset(
                                vp[:, jt, :, DH:128].bitcast(
                                    mybir.dt.uint32),
                                0x3F800000)

                    # ---- QK part: qkT [e, n] feature-major ----
                    for et in range(QK_TILES):
                        for ncn in range(2):
                            ps = psA.tile([128, 512], F32, tag="psA")
                            for dc in range(D_CHUNKS):
                                nc.tensor.matmul(
                                    ps[:],
                                    wqk[:, dc, et * 128:(et + 1) * 128],
                                    xT[:, dc, ncn * 512:(ncn + 1) * 512],
                                    start=(dc == 0),
                                    stop=(dc == D_CHUNKS - 1),
                                )
                            nc.vector.tensor_copy(
                                qkT[:, et, ncn * 512:(ncn + 1) * 512],
                                ps[:])

                if "b" not in phases:
                    # timing variant: consume tiles so DCE keeps the work
                    with tc.tile_pool(name="consume", bufs=1) as consume:
                        ct1 = consume.tile([128, 640], F32)
                        nc.vector.tensor_copy(
                            ct1[:, 0:512], qkT[:, 0, 0:512].bitcast(F32))
                        nc.vector.tensor_copy(
                            ct1[:, 512:640], vp[:, 0, 0, :].bitcast(F32))
                        nc.sync.dma_start(y_d.ap()[0:128, 0:640], ct1[:])
                    continue

                # ================= Phase B: attention =================
                with (
                    tc.tile_pool(name="cpool", bufs=1) as cpool,
                ):
                    wp = cpool.tile([128, D_CHUNKS, DIM], F32R)    # 18K/part
                    nc.sync.dma_start(
                        wp[:], wp_d.ap().rearrange("(dc p) f -> p dc f",
                                                   p=128))

                    # v1psum revert: psC alongside, psO single-buffered
                    from contextlib import ExitStack
                    _bstack = ExitStack()
                    if v1psum:
                        psC = _bstack.enter_context(
                            tc.tile_pool(name="psC", bufs=2, space="PSUM"))
                    with (
                        tc.tile_pool(name="ptpool", bufs=3) as ptpool,
                        tc.tile_pool(name="recpool", bufs=2) as recpool,
                        tc.tile_pool(name="psS", bufs=2, space="PSUM") as psS,
                        tc.tile_pool(name="psO", bufs=1 if v1psum else 2,
                                     space="PSUM") as psO,
                    ):
                        def normalize(h, ps_o):
                            # reciprocal_approx_fast needs SBUF input at
                            # partition base 0 — normalize at base 0, DVE
                            # handles the out-base shift on the final mult
                            base = 64 * (h % 2)
                            den = recpool.tile([64, N], F32, tag="den")
                            nc.vector.tensor_copy(den[0:64, :],
                                                  ps_o[64:128, :])
                            rec = recpool.tile([64, N], F32, tag="rec")
                            nc.vector.reciprocal_approx_fast(
                                rec[0:64, :], den[0:64, :])
                            nc.vector.tensor_tensor(
                                outT[base:base + 64, h // 2, :],
                                ps_o[0:64, :], rec[0:64, :],
                                op=AluOpType.mult)

                        prev = None   # (h, ps_o) awaiting normalize
                        for h in range(H):
                            base = 64 * (h % 2)
                            q_et = h // 2
                            k_et = H // 2 + h // 2
                            ps_o = psO.tile([128, N], F32, tag="ps_o")
                            for jt in range(N_TILES):
                                ps_s = psS.tile([128, N], F32, tag="ps_s")
                                for ic in range(2):
                                    nc.tensor.matmul(
                                        ps_s[:, ic * 512:(ic + 1) * 512],
                                        qkT[base:base + DH, k_et,
                                            jt * 128:(jt + 1) * 128],
                                        qkT[base:base + DH, q_et,
                                            ic * 512:(ic + 1) * 512],
                                        start=True, stop=True,
                                    )
                                if fexp_mod and jt in FEXP_JT:
                                    # DVE fast-exp (ACT offload)
                                    pti = ptpool.tile([128, N], I32,
                                                      tag="pti", bufs=2)
                                    nc.vector.tensor_scalar(
                                        pti[:], ps_s[:], FEXP_A, FEXP_C,
                                        AluOpType.mult, AluOpType.add)
                                    pt = pti.bitcast(F32R)
                                else:
                                    pt = ptpool.tile([128, N], F32R,
                                                     tag="pt")
                                    nc.scalar.activation(
                                        pt[:], ps_s[:], EXP, scale=SCALE)
                                for ic in range(2):
                                    nc.tensor.matmul(
                                        ps_o[:, ic * 512:(ic + 1) * 512],
                                        vp[:, jt, h, :],
                                        pt[:, ic * 512:(ic + 1) * 512],
                                        start=(jt == 0),
                                        stop=(jt == N_TILES - 1),
                                    )
                                # software pipelining: emit the PREVIOUS
                                # head's normalize mid-body so its long-
                                # satisfied deps never stall this head's
                                # exp/AV stream at the engine level
                                if jt == 2 and prev is not None:
                                    normalize(*prev)
                                    prev = None
                            if nodefer:
                                normalize(h, ps_o)
                            else:
                                prev = (h, ps_o)
                        if prev is not None:
                            normalize(*prev)

                    # ============= Phase C: output projection =============
                    # psS/psO released above — psC reuses those PSUM banks
                    if "c" not in phases:
                        with tc.tile_pool(name="ypool0", bufs=1) as ypool0:
                            yt0 = ypool0.tile([128, DIM], F32, tag="yt")
                            nc.vector.tensor_copy(
                                yt0[:, 0:DIM], outT[:, 0, 0:DIM].bitcast(F32))
                            nc.sync.dma_start(y_d.ap()[0:128, :], yt0[:])
                        continue
                    with ExitStack() as _cstack:
                        ypool = _cstack.enter_context(
                            tc.tile_pool(name="ypool", bufs=2))
                        if not v1psum:
                            psC = _cstack.enter_context(
                                tc.tile_pool(name="psC", bufs=4,
                                             space="PSUM"))
                        for nt in range(N_TILES):
                            yt = ypool.tile([128, DIM], F32, tag="yt")
                            for fc, fw in ((0, 512), (512, 256)):
                                ps = psC.tile([128, fw], F32, tag="psC")
                                for dc in range(D_CHUNKS):
                                    nc.tensor.matmul(
                                        ps[:],
                                        outT[:, dc, nt * 128:(nt + 1) * 128],
                                        wp[:, dc, fc:fc + fw],
                                        start=(dc == 0),
                                        stop=(dc == D_CHUNKS - 1),
                                    )
                                nc.vector.tensor_tensor(
                                    yt[:, fc:fc + fw], ps[:],
                                    bias_bc[:, fc:fc + fw], op=AluOpType.add)
                            nc.sync.dma_start(
                                y_d.ap().rearrange("(nt p) f -> p nt f",
                                                   p=128)[:, nt, :],
                                yt[:])
                    _bstack.close()

    nc.compile()
    return nc


def _prep_inputs(x, w_qkv, w_proj, b_proj, qkv_bf16=False):
    x = np.ascontiguousarray(np.asarray(x, dtype=np.float32))
    w_qkv = np.asarray(w_qkv, dtype=np.float32)
    w_proj = np.asarray(w_proj, dtype=np.float32)
    b_proj = np.ascontiguousarray(np.asarray(b_proj, dtype=np.float32))

    xT = np.ascontiguousarray(x.transpose(0, 2, 1))              # [B, D, N]
    wqkT = np.ascontiguousarray(w_qkv[:E_QK].T)                  # [D, 2D]
    wvT = np.ascontiguousarray(w_qkv[E_QK:].T)                   # [D, D]
    wpT = np.ascontiguousarray(w_proj.T)                         # [D, D]
    if qkv_bf16:
        import ml_dtypes
        xT = xT.astype(ml_dtypes.bfloat16)
        wqkT = wqkT.astype(ml_dtypes.bfloat16)
        wvT = wvT.astype(ml_dtypes.bfloat16)
    per_core = {"xT": None, "wqkT": wqkT, "wvT": wvT, "wpT": wpT,
                "b_proj": b_proj}

    def core_map(b):
        m = dict(per_core)
        m["xT"] = xT[b]
        return m

    return [core_map(b) for b in range(NCORES)]


_NC_CACHE = None


def _get_nc():
    global _NC_CACHE
    if _NC_CACHE is None:
        _NC_CACHE = build_nc()
    return _NC_CACHE


def kernel(x, w_qkv, w_proj, b_proj):
    in_maps = _prep_inputs(x, w_qkv, w_proj, b_proj)
    res = run_bass_kernel_spmd(_get_nc(), in_maps,
                               core_ids=list(range(NCORES)))
    return np.stack([res.results[b]["y"] for b in range(NCORES)], axis=0)


# revision 6
# speedup vs baseline: 29.8456x; 1.1210x over previous
"""Multi-head attention (B=8, N=1024, DIM=768, H=12) on 8 Trainium2 cores.

Sharding: data-parallel over batch — core b computes batch element b.
Per-core kernel: qkv = x @ w_qkv^T; per-head softmax(q k^T / sqrt(dh)) @ v;
out proj + bias.  All matmuls in float32r (TF32-like) mode.

Layout (per core, x_b is [N, D]):
  - host supplies x^T [D, N], w_qkv^T (split qk / v parts), w_proj^T
  - QKV phase: qkT [e, n] feature-major for q,k; v token-major [n, dh]
    packed per head as lhsT tiles [128, 128] = [v_h | ones] (ones block
    yields softmax denominators for free during attn@V)
  - scoresT[j, i] = k^T(lhsT) x q^T(rhs) -> PSUM; ACT exp (scale folded);
    no max-subtraction (scores ~ N(0,1), max << 80)
  - attn@V: out'[0:64] = unnormalized out^T, out'[64:128] = denom
  - normalize: reciprocal_approx_fast + tensor_tensor mult -> outT [d, n]
  - proj: y[n, f] = outT(lhsT) x wprojT(rhs) + bias

HW-tuned schedule (defaults = fastest measured on-device config):
  - vp ones memset on gpsimd (Pool engine idle otherwise)
  - per-head normalize deferred one head (software pipelining)
  - all input DMAs on the SP HWDGE queue; PSUM: psA 4 banks /
    psS 2x2 + psO 1x2 + psC 2x1 banks.  The cost-model-preferred
    variants (ACT-queue weight DMAs, split xT DMA, psO double-
    buffering) all measured SLOWER on real hardware; `revert` flags
    keep both schedules buildable for future bisection.
"""

import numpy as np

import concourse.bass as bass
import concourse.mybir as mybir
import concourse.tile as tile
from concourse import bacc
from concourse.alu_op_type import AluOpType
from concourse.bass_utils import run_bass_kernel_spmd

B, N, DIM, H = 8, 1024, 768, 12
DH = DIM // H          # 64
E_QK = 2 * DIM         # 1536
E_V = DIM              # 768
SCALE = DH ** -0.5
NCORES = 8

F32 = mybir.dt.float32
F32R = mybir.dt.float32r

N_TILES = N // 128     # 8
D_CHUNKS = DIM // 128  # 6
QK_TILES = E_QK // 128  # 12
EXP = mybir.ActivationFunctionType.Exp

# Schraudolph fast-exp constants: exp(s*SCALE) ~= bitcast_f32(round(
# s * (2^23/ln2 * SCALE) + (127 - 0.0437)*2^23)).  ~1.8% mean rel err;
# applied to a subset of softmax tiles to offload the ACT engine.
FEXP_A = float(2 ** 23 / np.log(2) * SCALE)
FEXP_C = float((127.0 - 0.043677) * 2 ** 23)
I32 = mybir.dt.int32
FEXP_JT = (3, 4, 5)


TIME_REPS = 32         # in-NEFF repetitions used by test.py's timing


def build_nc(reps=1, phases="abc", qkv_dt=None, fexp_mod=0, hw_loop=0,
             revert=("v1psum", "spdma"), psa_bufs=4, evac_act=False):
    """hw_loop>0: wrap ONE rep body in a tc.For_i hardware loop executing
    `hw_loop` iterations (compile cost of 1 rep, runtime of hw_loop reps);
    `reps` is the python-unrolled count (ignored when hw_loop is set).
    revert: any of {'spdma','dvememset','nodefer','v1psum'} to undo one
    v2 schedule change (HW bisection).  psa_bufs: phase-A PSUM depth.
    evac_act: qkT PSUM evacuations on the (idle-in-A) ACT engine."""
    qkv_dt = qkv_dt or F32R
    spdma = "spdma" in revert          # all input DMAs on SP, xT unsplit
    dvememset = "dvememset" in revert  # vp ones memset on DVE per jt
    nodefer = "nodefer" in revert      # normalize at head end
    v1psum = "v1psum" in revert        # psO bufs=1, psC pool alongside
    nc = bacc.Bacc("TRN2", target_bir_lowering=False, debug=False,
                   num_devices=NCORES)

    xT_d = nc.dram_tensor("xT", [DIM, N], qkv_dt, kind="ExternalInput")
    wqk_d = nc.dram_tensor("wqkT", [DIM, E_QK], qkv_dt, kind="ExternalInput")
    wv_d = nc.dram_tensor("wvT", [DIM, E_V], qkv_dt, kind="ExternalInput")
    wp_d = nc.dram_tensor("wpT", [DIM, DIM], F32R, kind="ExternalInput")
    bias_d = nc.dram_tensor("b_proj", [DIM], F32, kind="ExternalInput")
    y_d = nc.dram_tensor("y", [N, DIM], F32, kind="ExternalOutput")

    from contextlib import nullcontext
    with tile.TileContext(nc) as tc:
      loop_ctx = tc.For_i(0, hw_loop) if hw_loop else nullcontext()
      with loop_ctx:
       for _rep in range(1 if hw_loop else reps):
        with tc.tile_pool(name="persist", bufs=1) as persist:
            # ---- persistent tiles (live through proj) ----
            outT = persist.tile([128, D_CHUNKS, N], F32R)      # 24K/part
            bias_bc = persist.tile([128, DIM], F32)            # 3K/part

            nc.gpsimd.dma_start(
                out=bias_bc[:],
                in_=bias_d.ap()[None, :].broadcast_to([128, DIM]),
            )

            with tc.tile_pool(name="qkv_sb", bufs=1) as qkv_sb:
                # ---- tiles live until end of attention ----
                qkT = qkv_sb.tile([128, QK_TILES, N], F32R)        # 48K/part
                vp = qkv_sb.tile([128, N_TILES, H, 128], F32R)     # 48K/part

                # ================= Phase A: QKV projections =================
                with (
                    tc.tile_pool(name="xpool", bufs=1) as xpool,
                    tc.tile_pool(name="wvpool", bufs=1) as wvpool,
                    tc.tile_pool(name="wqkpool", bufs=1) as wqkpool,
                    tc.tile_pool(name="psA", bufs=psa_bufs,
                                 space="PSUM") as psA,
                ):
                    # SP queue: xT in 2 n-chunks (V/QK start on chunk 0)
                    xT = xpool.tile([128, D_CHUNKS, N], qkv_dt)  # 24K/part
                    xr = xT_d.ap().rearrange("(p dc) n -> p dc n",
                                             dc=D_CHUNKS)
                    if spdma:
                        nc.sync.dma_start(xT[:], xr)
                    else:
                        nc.sync.dma_start(xT[:, :, 0:512], xr[:, :, 0:512])
                        nc.sync.dma_start(xT[:, :, 512:N], xr[:, :, 512:N])
                    # ACT queue: wv then wqk then wp (all during phase A)
                    weng = nc.sync if spdma else nc.scalar
                    wv = wvpool.tile([128, D_CHUNKS, E_V], qkv_dt)
                    weng.dma_start(
                        wv[:],
                        wv_d.ap().rearrange("(p dc) f -> p dc f",
                                            dc=D_CHUNKS))
                    wqk = wqkpool.tile([128, D_CHUNKS, E_QK], qkv_dt)
                    weng.dma_start(
                        wqk[:],
                        wqk_d.ap().rearrange("(p dc) e -> p dc e",
                                             dc=D_CHUNKS))

                    # ---- V part: v[n, dh] per head + ones block ----
                    # ones block via gpsimd (idle Pool engine); memset on
                    # f32r fails walrus ISA check; write the 1.0f bit
                    # pattern through a uint32 view
                    if not dvememset:
                        nc.gpsimd.memset(
                            vp[:, :, :, DH:128].bitcast(mybir.dt.uint32),
                            0x3F800000)
                    for jt in range(N_TILES):
                        for fc, fw in ((0, 512), (512, 256)):
                            ps = psA.tile([128, fw], F32, tag="psA")
                            for dc in range(D_CHUNKS):
                                nc.tensor.matmul(
                                    ps[:],
                                    xT[:, dc, jt * 128:(jt + 1) * 128],
                                    wv[:, dc, fc:fc + fw],
                                    start=(dc == 0),
                                    stop=(dc == D_CHUNKS - 1),
                                )
                            h0, nh = fc // DH, fw // DH
                            nc.vector.tensor_copy(
                                vp[:, jt, h0:h0 + nh, 0:DH],
                                ps[:].rearrange("p (h c) -> p h c", c=DH),
                            )
                        if dvememset:
                            nc.vector.memset(
                                vp[:, jt, :, DH:128].bitcast(
                                    mybir.dt.uint32),
                                0x3F800000)

                    # ---- QK part: qkT [e, n] feature-major ----
                    for et in range(QK_TILES):
                        for ncn in range(2):
                            ps = psA.tile([128, 512], F32, tag="psA")
                            for dc in range(D_CHUNKS):
                                nc.tensor.matmul(
                                    ps[:],
                                    wqk[:, dc, et * 128:(et + 1) * 128],
                                    xT[:, dc, ncn * 512:(ncn + 1) * 512],
                                    start=(dc == 0),
                                    stop=(dc == D_CHUNKS - 1),
                                )
                            if evac_act:
                                nc.scalar.copy(
                                    qkT[:, et, ncn * 512:(ncn + 1) * 512],
                                    ps[:])
                            else:
                                nc.vector.tensor_copy(
                                    qkT[:, et, ncn * 512:(ncn + 1) * 512],
                                    ps[:])

                if "b" not in phases:
                    # timing variant: consume tiles so DCE keeps the work
                    with tc.tile_pool(name="consume", bufs=1) as consume:
                        ct1 = consume.tile([128, 640], F32)
                        nc.vector.tensor_copy(
                            ct1[:, 0:512], qkT[:, 0, 0:512].bitcast(F32))
                        nc.vector.tensor_copy(
                            ct1[:, 512:640], vp[:, 0, 0, :].bitcast(F32))
                        nc.sync.dma_start(y_d.ap()[0:128, 0:640], ct1[:])
                    continue

                # ================= Phase B: attention =================
                with (
                    tc.tile_pool(name="cpool", bufs=1) as cpool,
                ):
                    wp = cpool.tile([128, D_CHUNKS, DIM], F32R)    # 18K/part
                    nc.sync.dma_start(
                        wp[:], wp_d.ap().rearrange("(dc p) f -> p dc f",
                                                   p=128))

                    # v1psum revert: psC alongside, psO single-buffered
                    from contextlib import ExitStack
                    _bstack = ExitStack()
                    if v1psum:
                        psC = _bstack.enter_context(
                            tc.tile_pool(name="psC", bufs=2, space="PSUM"))
                    with (
                        tc.tile_pool(name="ptpool", bufs=3) as ptpool,
                        tc.tile_pool(name="recpool", bufs=2) as recpool,
                        tc.tile_pool(name="psS", bufs=2, space="PSUM") as psS,
                        tc.tile_pool(name="psO", bufs=1 if v1psum else 2,
                                     space="PSUM") as psO,
                    ):
                        def normalize(h, ps_o):
                            # reciprocal_approx_fast needs SBUF input at
                            # partition base 0 — normalize at base 0, DVE
                            # handles the out-base shift on the final mult
                            base = 64 * (h % 2)
                            den = recpool.tile([64, N], F32, tag="den")
                            nc.vector.tensor_copy(den[0:64, :],
                                                  ps_o[64:128, :])
                            rec = recpool.tile([64, N], F32, tag="rec")
                            nc.vector.reciprocal_approx_fast(
                                rec[0:64, :], den[0:64, :])
                            nc.vector.tensor_tensor(
                                outT[base:base + 64, h // 2, :],
                                ps_o[0:64, :], rec[0:64, :],
                                op=AluOpType.mult)

                        prev = None   # (h, ps_o) awaiting normalize
                        for h in range(H):
                            base = 64 * (h % 2)
                            q_et = h // 2
                            k_et = H // 2 + h // 2
                            ps_o = psO.tile([128, N], F32, tag="ps_o")
                            for jt in range(N_TILES):
                                ps_s = psS.tile([128, N], F32, tag="ps_s")
                                for ic in range(2):
                                    nc.tensor.matmul(
                                        ps_s[:, ic * 512:(ic + 1) * 512],
                                        qkT[base:base + DH, k_et,
                                            jt * 128:(jt + 1) * 128],
                                        qkT[base:base + DH, q_et,
                                            ic * 512:(ic + 1) * 512],
                                        start=True, stop=True,
                                    )
                                if fexp_mod and jt in FEXP_JT:
                                    # DVE fast-exp (ACT offload)
                                    pti = ptpool.tile([128, N], I32,
                                                      tag="pti", bufs=2)
                                    nc.vector.tensor_scalar(
                                        pti[:], ps_s[:], FEXP_A, FEXP_C,
                                        AluOpType.mult, AluOpType.add)
                                    pt = pti.bitcast(F32R)
                                else:
                                    pt = ptpool.tile([128, N], F32R,
                                                     tag="pt")
                                    nc.scalar.activation(
                                        pt[:], ps_s[:], EXP, scale=SCALE)
                                for ic in range(2):
                                    nc.tensor.matmul(
                                        ps_o[:, ic * 512:(ic + 1) * 512],
                                        vp[:, jt, h, :],
                                        pt[:, ic * 512:(ic + 1) * 512],
                                        start=(jt == 0),
                                        stop=(jt == N_TILES - 1),
                                    )
                                # software pipelining: emit the PREVIOUS
                                # head's normalize mid-body so its long-
                                # satisfied deps never stall this head's
                                # exp/AV stream at the engine level
                                if jt == 2 and prev is not None:
                                    normalize(*prev)
                                    prev = None
                            if nodefer:
                                normalize(h, ps_o)
                            else:
                                prev = (h, ps_o)
                        if prev is not None:
                            normalize(*prev)

                    # ============= Phase C: output projection =============
                    # psS/psO released above — psC reuses those PSUM banks
                    if "c" not in phases:
                        with tc.tile_pool(name="ypool0", bufs=1) as ypool0:
                            yt0 = ypool0.tile([128, DIM], F32, tag="yt")
                            nc.vector.tensor_copy(
                                yt0[:, 0:DIM], outT[:, 0, 0:DIM].bitcast(F32))
                            nc.sync.dma_start(y_d.ap()[0:128, :], yt0[:])
                        continue
                    with ExitStack() as _cstack:
                        ypool = _cstack.enter_context(
                            tc.tile_pool(name="ypool", bufs=2))
                        if not v1psum:
                            psC = _cstack.enter_context(
                                tc.tile_pool(name="psC", bufs=4,
                                             space="PSUM"))
                        for nt in range(N_TILES):
                            yt = ypool.tile([128, DIM], F32, tag="yt")
                            for fc, fw in ((0, 512), (512, 256)):
                                ps = psC.tile([128, fw], F32, tag="psC")
                                for dc in range(D_CHUNKS):
                                    nc.tensor.matmul(
                                        ps[:],
                                        outT[:, dc, nt * 128:(nt + 1) * 128],
                                        wp[:, dc, fc:fc + fw],
                                        start=(dc == 0),
                                        stop=(dc == D_CHUNKS - 1),
                                    )
                                nc.vector.tensor_tensor(
                                    yt[:, fc:fc + fw], ps[:],
                                    bias_bc[:, fc:fc + fw], op=AluOpType.add)
                            nc.sync.dma_start(
                                y_d.ap().rearrange("(nt p) f -> p nt f",
                                                   p=128)[:, nt, :],
                                yt[:])
                    _bstack.close()

    nc.compile()
    return nc


def _prep_inputs(x, w_qkv, w_proj, b_proj, qkv_bf16=False):
    x = np.ascontiguousarray(np.asarray(x, dtype=np.float32))
    w_qkv = np.asarray(w_qkv, dtype=np.float32)
    w_proj = np.asarray(w_proj, dtype=np.float32)
    b_proj = np.ascontiguousarray(np.asarray(b_proj, dtype=np.float32))

    xT = np.ascontiguousarray(x.transpose(0, 2, 1))              # [B, D, N]
    wqkT = np.ascontiguousarray(w_qkv[:E_QK].T)                  # [D, 2D]
    wvT = np.ascontiguousarray(w_qkv[E_QK:].T)                   # [D, D]
    wpT = np.ascontiguousarray(w_proj.T)                         # [D, D]
    if qkv_bf16:
        import ml_dtypes
        xT = xT.astype(ml_dtypes.bfloat16)
        wqkT = wqkT.astype(ml_dtypes.bfloat16)
        wvT = wvT.astype(ml_dtypes.bfloat16)
    per_core = {"xT": None, "wqkT": wqkT, "wvT": wvT, "wpT": wpT,
                "b_proj": b_proj}

    def core_map(b):
        m = dict(per_core)
        m["xT"] = xT[b]
        return m

    return [core_map(b) for b in range(NCORES)]


_NC_CACHE = None


def _get_nc():
    global _NC_CACHE
    if _NC_CACHE is None:
        _NC_CACHE = build_nc()
    return _NC_CACHE


def kernel(x, w_qkv, w_proj, b_proj):
    in_maps = _prep_inputs(x, w_qkv, w_proj, b_proj)
    res = run_bass_kernel_spmd(_get_nc(), in_maps,
                               core_ids=list(range(NCORES)))
    return np.stack([res.results[b]["y"] for b in range(NCORES)], axis=0)


# revision 8
# speedup vs baseline: 31.8844x; 1.0683x over previous
"""Multi-head attention (B=8, N=1024, DIM=768, H=12) on 8 Trainium2 cores.

Sharding: data-parallel over batch — core b computes batch element b.
Per-core kernel: qkv = x @ w_qkv^T; per-head softmax(q k^T / sqrt(dh)) @ v;
out proj + bias.  All matmuls in float32r (TF32-like) mode.

Layout (per core, x_b is [N, D]):
  - host supplies x^T [D, N], w_qkv^T (split qk / v parts), w_proj^T
  - QKV phase: qkT [e, n] feature-major for q,k; v token-major [n, dh]
    packed per head as lhsT tiles [128, 128] = [v_h | ones] (ones block
    yields softmax denominators for free during attn@V)
  - scoresT[j, i] = k^T(lhsT) x q^T(rhs) -> PSUM; ACT exp (scale folded);
    no max-subtraction (scores ~ N(0,1), max << 80)
  - attn@V: out'[0:64] = unnormalized out^T, out'[64:128] = denom
  - normalize: reciprocal_approx_fast + tensor_tensor mult -> outT [d, n]
  - proj: y[n, f] = outT(lhsT) x wprojT(rhs) + bias

HW-tuned schedule (defaults = fastest measured on-device config):
  - vp ones memset on gpsimd (Pool engine idle otherwise)
  - per-head normalize deferred one head (software pipelining)
  - all input DMAs on the SP HWDGE queue; PSUM: psA 4 banks /
    psS 2x2 + psO 1x2 + psC 2x1 banks.  The cost-model-preferred
    variants (ACT-queue weight DMAs, split xT DMA, psO double-
    buffering) all measured SLOWER on real hardware; `revert` flags
    keep both schedules buildable for future bisection.
"""

import numpy as np

import concourse.bass as bass
import concourse.mybir as mybir
import concourse.tile as tile
from concourse import bacc
from concourse.alu_op_type import AluOpType
from concourse.bass_utils import run_bass_kernel_spmd

B, N, DIM, H = 8, 1024, 768, 12
DH = DIM // H          # 64
E_QK = 2 * DIM         # 1536
E_V = DIM              # 768
SCALE = DH ** -0.5
NCORES = 8

F32 = mybir.dt.float32
F32R = mybir.dt.float32r

N_TILES = N // 128     # 8
D_CHUNKS = DIM // 128  # 6
QK_TILES = E_QK // 128  # 12
EXP = mybir.ActivationFunctionType.Exp

# Schraudolph fast-exp constants: exp(s*SCALE) ~= bitcast_f32(round(
# s * (2^23/ln2 * SCALE) + (127 - 0.0437)*2^23)).  ~1.8% mean rel err;
# applied to a subset of softmax tiles to offload the ACT engine.
FEXP_A = float(2 ** 23 / np.log(2) * SCALE)
FEXP_C = float((127.0 - 0.043677) * 2 ** 23)
I32 = mybir.dt.int32
FEXP_JT = (3, 4, 5)


TIME_REPS = 32         # in-NEFF repetitions used by test.py's timing


def build_nc(reps=1, phases="abc", qkv_dt=None, fexp_mod=0, hw_loop=0,
             revert=("v1psum", "spdma"), psa_bufs=4, evac_act=False):
    """hw_loop>0: wrap ONE rep body in a tc.For_i hardware loop executing
    `hw_loop` iterations (compile cost of 1 rep, runtime of hw_loop reps);
    `reps` is the python-unrolled count (ignored when hw_loop is set).
    revert: any of {'spdma','dvememset','nodefer','v1psum'} to undo one
    v2 schedule change (HW bisection).  psa_bufs: phase-A PSUM depth.
    evac_act: qkT PSUM evacuations on the (idle-in-A) ACT engine.
    qkv inputs default to bf16: halves input DMA bytes (~3% on HW, PE
    rate unchanged); rel-err 3.5e-4 -> 3.8e-3, still ~5x under the
    2e-2 gate."""
    qkv_dt = qkv_dt or mybir.dt.bfloat16
    spdma = "spdma" in revert          # all input DMAs on SP, xT unsplit
    dvememset = "dvememset" in revert  # vp ones memset on DVE per jt
    nodefer = "nodefer" in revert      # normalize at head end
    v1psum = "v1psum" in revert        # psO bufs=1, psC pool alongside
    nc = bacc.Bacc("TRN2", target_bir_lowering=False, debug=False,
                   num_devices=NCORES)

    xT_d = nc.dram_tensor("xT", [DIM, N], qkv_dt, kind="ExternalInput")
    wqk_d = nc.dram_tensor("wqkT", [DIM, E_QK], qkv_dt, kind="ExternalInput")
    wv_d = nc.dram_tensor("wvT", [DIM, E_V], qkv_dt, kind="ExternalInput")
    wp_d = nc.dram_tensor("wpT", [DIM, DIM], F32R, kind="ExternalInput")
    bias_d = nc.dram_tensor("b_proj", [DIM], F32, kind="ExternalInput")
    y_d = nc.dram_tensor("y", [N, DIM], F32, kind="ExternalOutput")

    from contextlib import nullcontext
    with tile.TileContext(nc) as tc:
      loop_ctx = tc.For_i(0, hw_loop) if hw_loop else nullcontext()
      with loop_ctx:
       for _rep in range(1 if hw_loop else reps):
        with tc.tile_pool(name="persist", bufs=1) as persist:
            # ---- persistent tiles (live through proj) ----
            outT = persist.tile([128, D_CHUNKS, N], F32R)      # 24K/part
            bias_bc = persist.tile([128, DIM], F32)            # 3K/part

            nc.gpsimd.dma_start(
                out=bias_bc[:],
                in_=bias_d.ap()[None, :].broadcast_to([128, DIM]),
            )

            with tc.tile_pool(name="qkv_sb", bufs=1) as qkv_sb:
                # ---- tiles live until end of attention ----
                qkT = qkv_sb.tile([128, QK_TILES, N], F32R)        # 48K/part
                vp = qkv_sb.tile([128, N_TILES, H, 128], F32R)     # 48K/part

                # ================= Phase A: QKV projections =================
                with (
                    tc.tile_pool(name="xpool", bufs=1) as xpool,
                    tc.tile_pool(name="wvpool", bufs=1) as wvpool,
                    tc.tile_pool(name="wqkpool", bufs=1) as wqkpool,
                    tc.tile_pool(name="psA", bufs=psa_bufs,
                                 space="PSUM") as psA,
                ):
                    # SP queue: xT in 2 n-chunks (V/QK start on chunk 0)
                    xT = xpool.tile([128, D_CHUNKS, N], qkv_dt)  # 24K/part
                    xr = xT_d.ap().rearrange("(p dc) n -> p dc n",
                                             dc=D_CHUNKS)
                    if spdma:
                        nc.sync.dma_start(xT[:], xr)
                    else:
                        nc.sync.dma_start(xT[:, :, 0:512], xr[:, :, 0:512])
                        nc.sync.dma_start(xT[:, :, 512:N], xr[:, :, 512:N])
                    # ACT queue: wv then wqk then wp (all during phase A)
                    weng = nc.sync if spdma else nc.scalar
                    wv = wvpool.tile([128, D_CHUNKS, E_V], qkv_dt)
                    weng.dma_start(
                        wv[:],
                        wv_d.ap().rearrange("(p dc) f -> p dc f",
                                            dc=D_CHUNKS))
                    wqk = wqkpool.tile([128, D_CHUNKS, E_QK], qkv_dt)
                    weng.dma_start(
                        wqk[:],
                        wqk_d.ap().rearrange("(p dc) e -> p dc e",
                                             dc=D_CHUNKS))

                    # ---- V part: v[n, dh] per head + ones block ----
                    # ones block via gpsimd (idle Pool engine); memset on
                    # f32r fails walrus ISA check; write the 1.0f bit
                    # pattern through a uint32 view
                    if not dvememset:
                        nc.gpsimd.memset(
                            vp[:, :, :, DH:128].bitcast(mybir.dt.uint32),
                            0x3F800000)
                    for jt in range(N_TILES):
                        for fc, fw in ((0, 512), (512, 256)):
                            ps = psA.tile([128, fw], F32, tag="psA")
                            for dc in range(D_CHUNKS):
                                nc.tensor.matmul(
                                    ps[:],
                                    xT[:, dc, jt * 128:(jt + 1) * 128],
                                    wv[:, dc, fc:fc + fw],
                                    start=(dc == 0),
                                    stop=(dc == D_CHUNKS - 1),
                                )
                            h0, nh = fc // DH, fw // DH
                            nc.vector.tensor_copy(
                                vp[:, jt, h0:h0 + nh, 0:DH],
                                ps[:].rearrange("p (h c) -> p h c", c=DH),
                            )
                        if dvememset:
                            nc.vector.memset(
                                vp[:, jt, :, DH:128].bitcast(
                                    mybir.dt.uint32),
                                0x3F800000)

                    # ---- QK part: qkT [e, n] feature-major ----
                    for et in range(QK_TILES):
                        for ncn in range(2):
                            ps = psA.tile([128, 512], F32, tag="psA")
                            for dc in range(D_CHUNKS):
                                nc.tensor.matmul(
                                    ps[:],
                                    wqk[:, dc, et * 128:(et + 1) * 128],
                                    xT[:, dc, ncn * 512:(ncn + 1) * 512],
                                    start=(dc == 0),
                                    stop=(dc == D_CHUNKS - 1),
                                )
                            if evac_act:
                                nc.scalar.copy(
                                    qkT[:, et, ncn * 512:(ncn + 1) * 512],
                                    ps[:])
                            else:
                                nc.vector.tensor_copy(
                                    qkT[:, et, ncn * 512:(ncn + 1) * 512],
                                    ps[:])

                if "b" not in phases:
                    # timing variant: consume tiles so DCE keeps the work
                    with tc.tile_pool(name="consume", bufs=1) as consume:
                        ct1 = consume.tile([128, 640], F32)
                        nc.vector.tensor_copy(
                            ct1[:, 0:512], qkT[:, 0, 0:512].bitcast(F32))
                        nc.vector.tensor_copy(
                            ct1[:, 512:640], vp[:, 0, 0, :].bitcast(F32))
                        nc.sync.dma_start(y_d.ap()[0:128, 0:640], ct1[:])
                    continue

                # ================= Phase B: attention =================
                with (
                    tc.tile_pool(name="cpool", bufs=1) as cpool,
                ):
                    wp = cpool.tile([128, D_CHUNKS, DIM], F32R)    # 18K/part
                    nc.sync.dma_start(
                        wp[:], wp_d.ap().rearrange("(dc p) f -> p dc f",
                                                   p=128))

                    # v1psum revert: psC alongside, psO single-buffered
                    from contextlib import ExitStack
                    _bstack = ExitStack()
                    if v1psum:
                        psC = _bstack.enter_context(
                            tc.tile_pool(name="psC", bufs=2, space="PSUM"))
                    with (
                        tc.tile_pool(name="ptpool", bufs=3) as ptpool,
                        tc.tile_pool(name="recpool", bufs=2) as recpool,
                        tc.tile_pool(name="psS", bufs=2, space="PSUM") as psS,
                        tc.tile_pool(name="psO", bufs=1 if v1psum else 2,
                                     space="PSUM") as psO,
                    ):
                        def normalize(h, ps_o):
                            # reciprocal_approx_fast needs SBUF input at
                            # partition base 0 — normalize at base 0, DVE
                            # handles the out-base shift on the final mult
                            base = 64 * (h % 2)
                            den = recpool.tile([64, N], F32, tag="den")
                            nc.vector.tensor_copy(den[0:64, :],
                                                  ps_o[64:128, :])
                            rec = recpool.tile([64, N], F32, tag="rec")
                            nc.vector.reciprocal_approx_fast(
                                rec[0:64, :], den[0:64, :])
                            nc.vector.tensor_tensor(
                                outT[base:base + 64, h // 2, :],
                                ps_o[0:64, :], rec[0:64, :],
                                op=AluOpType.mult)

                        prev = None   # (h, ps_o) awaiting normalize
                        for h in range(H):
                            base = 64 * (h % 2)
                            q_et = h // 2
                            k_et = H // 2 + h // 2
                            ps_o = psO.tile([128, N], F32, tag="ps_o")
                            for jt in range(N_TILES):
                                ps_s = psS.tile([128, N], F32, tag="ps_s")
                                for ic in range(2):
                                    nc.tensor.matmul(
                                        ps_s[:, ic * 512:(ic + 1) * 512],
                                        qkT[base:base + DH, k_et,
                                            jt * 128:(jt + 1) * 128],
                                        qkT[base:base + DH, q_et,
                                            ic * 512:(ic + 1) * 512],
                                        start=True, stop=True,
                                    )
                                if fexp_mod and jt in FEXP_JT:
                                    # DVE fast-exp (ACT offload)
                                    pti = ptpool.tile([128, N], I32,
                                                      tag="pti", bufs=2)
                                    nc.vector.tensor_scalar(
                                        pti[:], ps_s[:], FEXP_A, FEXP_C,
                                        AluOpType.mult, AluOpType.add)
                                    pt = pti.bitcast(F32R)
                                else:
                                    pt = ptpool.tile([128, N], F32R,
                                                     tag="pt")
                                    nc.scalar.activation(
                                        pt[:], ps_s[:], EXP, scale=SCALE)
                                for ic in range(2):
                                    nc.tensor.matmul(
                                        ps_o[:, ic * 512:(ic + 1) * 512],
                                        vp[:, jt, h, :],
                                        pt[:, ic * 512:(ic + 1) * 512],
                                        start=(jt == 0),
                                        stop=(jt == N_TILES - 1),
                                    )
                                # software pipelining: emit the PREVIOUS
                                # head's normalize mid-body so its long-
                                # satisfied deps never stall this head's
                                # exp/AV stream at the engine level
                                if jt == 2 and prev is not None:
                                    normalize(*prev)
                                    prev = None
                            if nodefer:
                                normalize(h, ps_o)
                            else:
                                prev = (h, ps_o)
                        if prev is not None:
                            normalize(*prev)

                    # ============= Phase C: output projection =============
                    # psS/psO released above — psC reuses those PSUM banks
                    if "c" not in phases:
                        with tc.tile_pool(name="ypool0", bufs=1) as ypool0:
                            yt0 = ypool0.tile([128, DIM], F32, tag="yt")
                            nc.vector.tensor_copy(
                                yt0[:, 0:DIM], outT[:, 0, 0:DIM].bitcast(F32))
                            nc.sync.dma_start(y_d.ap()[0:128, :], yt0[:])
                        continue
                    with ExitStack() as _cstack:
                        ypool = _cstack.enter_context(
                            tc.tile_pool(name="ypool", bufs=2))
                        if not v1psum:
                            psC = _cstack.enter_context(
                                tc.tile_pool(name="psC", bufs=4,
                                             space="PSUM"))
                        for nt in range(N_TILES):
                            yt = ypool.tile([128, DIM], F32, tag="yt")
                            for fc, fw in ((0, 512), (512, 256)):
                                ps = psC.tile([128, fw], F32, tag="psC")
                                for dc in range(D_CHUNKS):
                                    nc.tensor.matmul(
                                        ps[:],
                                        outT[:, dc, nt * 128:(nt + 1) * 128],
                                        wp[:, dc, fc:fc + fw],
                                        start=(dc == 0),
                                        stop=(dc == D_CHUNKS - 1),
                                    )
                                nc.vector.tensor_tensor(
                                    yt[:, fc:fc + fw], ps[:],
                                    bias_bc[:, fc:fc + fw], op=AluOpType.add)
                            nc.sync.dma_start(
                                y_d.ap().rearrange("(nt p) f -> p nt f",
                                                   p=128)[:, nt, :],
                                yt[:])
                    _bstack.close()

    nc.compile()
    return nc


def _prep_inputs(x, w_qkv, w_proj, b_proj, qkv_bf16=True):
    x = np.ascontiguousarray(np.asarray(x, dtype=np.float32))
    w_qkv = np.asarray(w_qkv, dtype=np.float32)
    w_proj = np.asarray(w_proj, dtype=np.float32)
    b_proj = np.ascontiguousarray(np.asarray(b_proj, dtype=np.float32))

    xT = np.ascontiguousarray(x.transpose(0, 2, 1))              # [B, D, N]
    wqkT = np.ascontiguousarray(w_qkv[:E_QK].T)                  # [D, 2D]
    wvT = np.ascontiguousarray(w_qkv[E_QK:].T)                   # [D, D]
    wpT = np.ascontiguousarray(w_proj.T)                         # [D, D]
    if qkv_bf16:
        import ml_dtypes
        xT = xT.astype(ml_dtypes.bfloat16)
        wqkT = wqkT.astype(ml_dtypes.bfloat16)
        wvT = wvT.astype(ml_dtypes.bfloat16)
    per_core = {"xT": None, "wqkT": wqkT, "wvT": wvT, "wpT": wpT,
                "b_proj": b_proj}

    def core_map(b):
        m = dict(per_core)
        m["xT"] = xT[b]
        return m

    return [core_map(b) for b in range(NCORES)]


_NC_CACHE = None


def _get_nc():
    global _NC_CACHE
    if _NC_CACHE is None:
        _NC_CACHE = build_nc()
    return _NC_CACHE


def kernel(x, w_qkv, w_proj, b_proj):
    in_maps = _prep_inputs(x, w_qkv, w_proj, b_proj)
    res = run_bass_kernel_spmd(_get_nc(), in_maps,
                               core_ids=list(range(NCORES)))
    return np.stack([res.results[b]["y"] for b in range(NCORES)], axis=0)
